# revision 31
# baseline (speedup 1.0000x reference)
"""DotProductAttentionPooling on 8 trn2 NeuronCores.

reference:
    scores = einsum("bld,d->bl", x, q) / sqrt(D)
    scores = where(mask, scores, -inf)
    attn   = nan_to_num(softmax(scores, axis=-1))
    out    = einsum("bl,bld->bd", attn, x)            # [B, D]

Strategy (memory-bound: x is 256 MiB and must be read exactly once):
  - Data-parallel: batch B=32 sharded 4-per-core across 8 cores; query
    replicated; output [B, D] gathered on host.
  - x[b] streams to SBUF in natural layout [128(L-part), chunk, 256(D)]
    with l = p*64 + i so each partition's HBM read is one contiguous
    64 KiB run. Quarter-batch (16-chunk) tiles pipeline DMA / compute;
    DMA issues are software-pipelined over a global quarter index so
    the sync sequencer spreads descriptor pushes evenly instead of
    bursting at batch boundaries.
  - Scores: one fused DVE scalar_tensor_tensor per [128, 256] chunk
    straight off the fp32 DMA tiles (fp32-accurate):
    scr = (x * 1/sqrt(D)) * q, accum_out = row-sum -> scores column.
    The last NOFF chunks per quarter instead run as a 2x-rate fp16
    tensor_tensor on DVE plus a scaled activation-accum reduce on
    ScalarE, shaving the DVE critical path.
  - ScalarE converts each quarter to fp16 (one quarter ahead of use so
    the convert never waits behind exp) for the TensorE pooling matmul
    (fp16 1-pass vs fp32's 2-pass). All accumulation stays fp32.
  - Softmax without max-subtraction: scores are O(0.3) so exp cannot
    overflow; the -inf mask becomes w = exp(scores) * mask. exp, mask
    multiply and pooling run per quarter so pooling starts before the
    batch finishes; denominator = ones-matmul over the per-quarter
    row-sums, +1e-30 so an all-masked batch yields 0 (like
    nan_to_num), not NaN.
  - Pooling: unnormalized acc[1, 256] += w_col.T @ x_chunk as fp16
    accumulating PE matmuls (contraction over partition dim = L);
    final normalize on ScalarE out of PSUM, deferred one batch so it
    never stalls the pipeline.
"""

import numpy as np

B, L, D = 32, 8192, 256
N_CORES = 8
BPC = B // N_CORES        # batches per core
P = 128                   # partitions
CHUNKS = L // P           # 64 L-chunks per batch
QC = 16                   # chunks per quarter tile
NQ = CHUNKS // QC         # quarters per batch
NQT = BPC * NQ            # total quarters per core
LOOKAHEAD = 8             # quarters of DMA prefetch (= xf32 bufs)
NOFF = 3                  # chunks per quarter offloaded to fp16-TT+ScalarE
SCALE = 1.0 / float(np.sqrt(D))

_cache = {}


def _build():
    import concourse.bacc as bacc
    import concourse.bass as bass
    import concourse.tile as tile
    from concourse import mybir

    f32 = mybir.dt.float32
    f16 = mybir.dt.float16
    i32 = mybir.dt.int32
    nc = bacc.Bacc("TRN2", target_bir_lowering=False, debug=False,
                   num_devices=N_CORES)

    x = nc.declare_dram_parameter("x", [BPC, L, D], f32, isOutput=False)
    mask = nc.declare_dram_parameter("mask", [BPC, L], i32, isOutput=False)
    query = nc.declare_dram_parameter("query", [D], f32, isOutput=False)
    out = nc.declare_dram_parameter("out", [BPC, D], f32, isOutput=True)

    # l = p * CHUNKS + i: per-partition HBM reads are contiguous
    x_r = x[:].rearrange("b (p i) d -> b p i d", p=P)
    mask_r = mask[:].rearrange("b (p i) -> b p i", p=P)

    with tile.TileContext(nc) as tc:
        with (
            tc.tile_pool(name="xf32", bufs=LOOKAHEAD) as xf32p,
            tc.tile_pool(name="xf16", bufs=6) as xf16p,
            tc.tile_pool(name="small", bufs=4) as small,
            tc.tile_pool(name="scratch", bufs=2) as scratchp,
            tc.tile_pool(name="singles", bufs=1) as singles,
            tc.tile_pool(name="psum", bufs=2, space="PSUM") as psums,
        ):
            # broadcast query across partitions with a step-0 SWDGE DMA,
            # issued first so it lands before the first score op needs it
            qb = singles.tile([P, D], f32)
            q_ap = query[:]
            nc.gpsimd.dma_start(out=qb[:], in_=bass.AP(
                tensor=q_ap.tensor, offset=q_ap.offset,
                ap=[[0, P]] + list(q_ap.ap)))
            qh = singles.tile([P, D], f16)
            nc.scalar.copy(qh[:], qb[:])
            ones = singles.tile([P, 1], f32)
            nc.vector.memset(ones[:], 1.0)

            xq_tiles = {}       # quarter index -> staged fp32 tile
            xh_tiles = {}       # quarter index -> fp16 copy for TensorE
            mask_tiles = {}     # batch -> int32 mask tile
            state = {}          # per-batch softmax state
            epilogue = []       # deferred (pool_ps, rden, b)

            def issue_quarter(k):
                b, qi = divmod(k, NQ)
                xq = xf32p.tile([P, QC, D], f32, tag="xf")
                # first quarters: finer DMA slices so the pipeline fills fast
                ndma = 4 if k == 0 else 1
                step = QC // ndma
                for g in range(ndma):
                    sl = slice(qi * QC + g * step, qi * QC + (g + 1) * step)
                    nc.sync.dma_start(out=xq[:, g * step:(g + 1) * step, :],
                                      in_=x_r[b, :, sl, :])
                xq_tiles[k] = xq
                if qi == 0:
                    mi = small.tile([P, CHUNKS], i32, tag="mask_i")
                    nc.sync.dma_start(out=mi[:], in_=mask_r[b])
                    mask_tiles[b] = mi

            def flush_epilogue():
                while epilogue:
                    pool_ps, wsum4, bb = epilogue.pop()
                    den_ps = psums.tile([1, NQ], f32, tag="den",
                                        name=f"den_ps{bb}")
                    nc.tensor.matmul(den_ps[:], ones[:], wsum4[:],
                                     start=True, stop=True)
                    den_sb = small.tile([1, 1], f32, tag="den_sb",
                                        name=f"den_sb{bb}")
                    nc.vector.tensor_reduce(out=den_sb[:], in_=den_ps[:],
                                            op=mybir.AluOpType.add,
                                            axis=mybir.AxisListType.X)
                    # +1e-30 so an all-masked batch divides to 0, not NaN
                    den_eps = small.tile([1, 1], f32, tag="den_eps",
                                         name=f"den_eps{bb}")
                    nc.vector.tensor_scalar_add(den_eps[:], den_sb[:], 1e-30)
                    rden = small.tile([1, 1], f32, tag="rden",
                                      name=f"rden{bb}")
                    nc.vector.reciprocal(rden[:], den_eps[:])
                    out_sb = small.tile([1, D], f32)
                    nc.scalar.activation(
                        out=out_sb[:], in_=pool_ps[:],
                        func=mybir.ActivationFunctionType.Copy,
                        scale=rden[0:1, 0:1])
                    nc.sync.dma_start(out=out[bb:bb + 1, :], in_=out_sb[:])

            for k in range(min(LOOKAHEAD, NQT)):
                issue_quarter(k)

            for k in range(NQT):
                b, qi = divmod(k, NQ)
                if qi == 0:
                    state[b] = {
                        "scores": small.tile([P, CHUNKS], f32, tag="scores",
                                             name=f"scores{b}"),
                        "wsum4": small.tile([P, NQ], f32, tag="wsum4",
                                            name=f"wsum4_{b}"),
                        "pool_ps": psums.tile([1, D], f32, tag="pool",
                                              name=f"pool_ps{b}"),
                        "mask_f": small.tile([P, CHUNKS], f32, tag="mask_f",
                                             name=f"mask_f{b}"),
                    }
                st = state[b]
                xq = xq_tiles.pop(k)

                # convert quarter k (if not already) and quarter k+1 ahead of
                # the STT stream so ScalarE's convert never waits behind exp
                if k == 0:
                    xh0 = xf16p.tile([P, QC, D], f16, tag="xh", name="xh0")
                    nc.scalar.copy(xh0[:], xq[:])
                    xh_tiles[0] = xh0
                xh = xh_tiles.pop(k)
                if k + 1 < NQT:
                    xh1 = xf16p.tile([P, QC, D], f16, tag="xh",
                                     name=f"xh{k + 1}")
                    nc.scalar.copy(xh1[:], xq_tiles[k + 1][:])
                    xh_tiles[k + 1] = xh1

                noff_k = 0 if k == NQT - 1 else NOFF
                for i in range(QC):
                    col = st["scores"][:, qi * QC + i:qi * QC + i + 1]
                    if i < QC - noff_k:
                        scr = scratchp.tile([P, D], f32, tag="scr")
                        nc.vector.scalar_tensor_tensor(
                            out=scr[:],
                            in0=xq[:, i, :],
                            scalar=SCALE,
                            in1=qb[:],
                            op0=mybir.AluOpType.mult,
                            op1=mybir.AluOpType.mult,
                            accum_out=col,
                        )
                    else:
                        # offload: fp16 product at DVE 2x rate, scaled
                        # reduce on ScalarE via activation accum
                        prodh = scratchp.tile([P, D], f16, tag="prodh",
                                              bufs=3)
                        nc.vector.tensor_tensor(
                            out=prodh[:], in0=xh[:, i, :], in1=qh[:],
                            op=mybir.AluOpType.mult)
                        scrh = scratchp.tile([P, D], f16, tag="scrh")
                        nc.scalar.activation(
                            out=scrh[:], in_=prodh[:],
                            func=mybir.ActivationFunctionType.Copy,
                            scale=SCALE, accum_out=col)

                if b > 0 and qi == 0:
                    flush_epilogue()
                if k + LOOKAHEAD < NQT:
                    issue_quarter(k + LOOKAHEAD)
                if qi == 0:
                    # mask cast deferred past the STT stream so the DVE's
                    # quarter start never waits on the mask DMA
                    nc.vector.tensor_copy(st["mask_f"][:],
                                          mask_tiles[b][:])

                # per-quarter softmax tail: exp, mask, fp16 weights
                sl = slice(qi * QC, (qi + 1) * QC)
                expq = small.tile([P, QC], f32, tag="expq")
                nc.scalar.activation(out=expq[:], in_=st["scores"][:, sl],
                                     func=mybir.ActivationFunctionType.Exp)
                wqh = small.tile([P, QC], f16, tag="wqh")
                nc.vector.scalar_tensor_tensor(
                    out=wqh[:], in0=expq[:], scalar=1.0,
                    in1=st["mask_f"][:, sl],
                    op0=mybir.AluOpType.mult, op1=mybir.AluOpType.mult,
                    accum_out=st["wsum4"][:, qi:qi + 1],
                )

                for i in range(QC):
                    nc.tensor.matmul(
                        st["pool_ps"][:],
                        wqh[:, i:i + 1],
                        xh[:, i, :],
                        start=(qi == 0 and i == 0),
                        stop=(qi == NQ - 1 and i == QC - 1),
                    )

                if qi == NQ - 1:
                    epilogue.append((st["pool_ps"], st["wsum4"], b))
                    del state[b]

            flush_epilogue()

    nc.compile()
    return nc


def kernel(x: np.ndarray, mask: np.ndarray, query: np.ndarray) -> np.ndarray:
    from concourse.bass_utils import run_bass_kernel_spmd

    if "nc" not in _cache:
        _cache["nc"] = _build()
    nc = _cache["nc"]

    x = np.ascontiguousarray(np.asarray(x, dtype=np.float32))
    mask = np.ascontiguousarray(np.asarray(mask, dtype=np.int32))
    query = np.ascontiguousarray(np.asarray(query, dtype=np.float32))

    in_maps = [
        {
            "x": np.ascontiguousarray(x[c * BPC:(c + 1) * BPC]),
            "mask": np.ascontiguousarray(mask[c * BPC:(c + 1) * BPC]),
            "query": query,
        }
        for c in range(N_CORES)
    ]
    res = run_bass_kernel_spmd(nc, in_maps, core_ids=list(range(N_CORES)))
    return np.concatenate([res.results[c]["out"] for c in range(N_CORES)], axis=0)


# revision 32
# speedup vs baseline: 1.1391x; 1.1391x over previous
"""DotProductAttentionPooling on 8 trn2 NeuronCores.

reference:
    scores = einsum("bld,d->bl", x, q) / sqrt(D)
    scores = where(mask, scores, -inf)
    attn   = nan_to_num(softmax(scores, axis=-1))
    out    = einsum("bl,bld->bd", attn, x)            # [B, D]

Strategy (memory-bound: x is 256 MiB and must be read exactly once):
  - Data-parallel: batch B=32 sharded 4-per-core across 8 cores; query
    replicated; output [B, D] gathered on host.
  - x[b] streams to SBUF in natural layout [128(L-part), chunk, 256(D)]
    with l = p*64 + i so each partition's HBM read is one contiguous
    64 KiB run. Quarter-batch (16-chunk) tiles pipeline DMA / compute;
    DMA issues are software-pipelined over a global quarter index so
    the sync sequencer spreads descriptor pushes evenly instead of
    bursting at batch boundaries.
  - Scores: one fused DVE scalar_tensor_tensor per [128, 256] chunk
    straight off the fp32 DMA tiles (fp32-accurate):
    scr = (x * 1/sqrt(D)) * q, accum_out = row-sum -> scores column.
    The last NOFF chunks per quarter instead run as a 2x-rate fp16
    tensor_tensor on DVE plus a scaled activation-accum reduce on
    ScalarE, shaving the DVE critical path.
  - ScalarE converts each quarter to fp16 (one quarter ahead of use so
    the convert never waits behind exp) for the TensorE pooling matmul
    (fp16 1-pass vs fp32's 2-pass). All accumulation stays fp32.
  - Softmax without max-subtraction: scores are O(0.3) so exp cannot
    overflow; the -inf mask becomes w = exp(scores) * mask. exp, mask
    multiply and pooling run per quarter so pooling starts before the
    batch finishes; denominator = ones-matmul over the per-quarter
    row-sums, +1e-30 so an all-masked batch yields 0 (like
    nan_to_num), not NaN.
  - Pooling: unnormalized acc[1, 256] += w_col.T @ x_chunk as fp16
    accumulating PE matmuls (contraction over partition dim = L);
    final normalize on ScalarE out of PSUM, deferred one batch so it
    never stalls the pipeline.
"""

import numpy as np

B, L, D = 32, 8192, 256
N_CORES = 8
BPC = B // N_CORES        # batches per core
P = 128                   # partitions
CHUNKS = L // P           # 64 L-chunks per batch
QC = 16                   # chunks per quarter tile
NQ = CHUNKS // QC         # quarters per batch
NQT = BPC * NQ            # total quarters per core
LOOKAHEAD = 8             # quarters of DMA prefetch (= xf32 bufs)
NOFF = 3                  # chunks per quarter offloaded to fp16-TT+ScalarE
SCALE = 1.0 / float(np.sqrt(D))

_cache = {}


def _build():
    import concourse.bacc as bacc
    import concourse.bass as bass
    import concourse.tile as tile
    from concourse import mybir

    f32 = mybir.dt.float32
    f16 = mybir.dt.float16
    i32 = mybir.dt.int32
    nc = bacc.Bacc("TRN2", target_bir_lowering=False, debug=False,
                   num_devices=N_CORES)

    x = nc.declare_dram_parameter("x", [BPC, L, D], f32, isOutput=False)
    mask = nc.declare_dram_parameter("mask", [BPC, L], i32, isOutput=False)
    query = nc.declare_dram_parameter("query", [D], f32, isOutput=False)
    out = nc.declare_dram_parameter("out", [BPC, D], f32, isOutput=True)

    # l = p * CHUNKS + i: per-partition HBM reads are contiguous
    x_r = x[:].rearrange("b (p i) d -> b p i d", p=P)
    mask_r = mask[:].rearrange("b (p i) -> b p i", p=P)

    with tile.TileContext(nc) as tc:
        with (
            tc.tile_pool(name="xf32", bufs=LOOKAHEAD) as xf32p,
            tc.tile_pool(name="xf16", bufs=6) as xf16p,
            tc.tile_pool(name="small", bufs=4) as small,
            tc.tile_pool(name="scratch", bufs=2) as scratchp,
            tc.tile_pool(name="singles", bufs=1) as singles,
            tc.tile_pool(name="psum", bufs=2, space="PSUM") as psums,
        ):
            # broadcast query across partitions with a step-0 SWDGE DMA,
            # issued first so it lands before the first score op needs it
            qb = singles.tile([P, D], f32)
            q_ap = query[:]
            nc.gpsimd.dma_start(out=qb[:], in_=bass.AP(
                tensor=q_ap.tensor, offset=q_ap.offset,
                ap=[[0, P]] + list(q_ap.ap)))
            qh = singles.tile([P, D], f16)
            nc.scalar.copy(qh[:], qb[:])
            ones = singles.tile([P, 1], f32)
            nc.vector.memset(ones[:], 1.0)

            xq_tiles = {}       # quarter index -> staged fp32 tile
            xh_tiles = {}       # quarter index -> fp16 copy for TensorE
            mask_tiles = {}     # batch -> int32 mask tile
            state = {}          # per-batch softmax state
            epilogue = []       # deferred (pool_ps, rden, b)

            def issue_quarter(k):
                b, qi = divmod(k, NQ)
                xq = xf32p.tile([P, QC, D], f32, tag="xf")
                # first quarters: finer DMA slices so the pipeline fills fast
                ndma = 4 if k == 0 else 2
                step = QC // ndma
                for g in range(ndma):
                    sl = slice(qi * QC + g * step, qi * QC + (g + 1) * step)
                    nc.sync.dma_start(out=xq[:, g * step:(g + 1) * step, :],
                                      in_=x_r[b, :, sl, :])
                xq_tiles[k] = xq
                if qi == 0:
                    mi = small.tile([P, CHUNKS], i32, tag="mask_i")
                    nc.sync.dma_start(out=mi[:], in_=mask_r[b])
                    mask_tiles[b] = mi

            def flush_epilogue():
                while epilogue:
                    pool_ps, wsum4, bb = epilogue.pop()
                    den_ps = psums.tile([1, NQ], f32, tag="den",
                                        name=f"den_ps{bb}")
                    nc.tensor.matmul(den_ps[:], ones[:], wsum4[:],
                                     start=True, stop=True)
                    den_sb = small.tile([1, 1], f32, tag="den_sb",
                                        name=f"den_sb{bb}")
                    nc.vector.tensor_reduce(out=den_sb[:], in_=den_ps[:],
                                            op=mybir.AluOpType.add,
                                            axis=mybir.AxisListType.X)
                    # +1e-30 so an all-masked batch divides to 0, not NaN
                    den_eps = small.tile([1, 1], f32, tag="den_eps",
                                         name=f"den_eps{bb}")
                    nc.vector.tensor_scalar_add(den_eps[:], den_sb[:], 1e-30)
                    rden = small.tile([1, 1], f32, tag="rden",
                                      name=f"rden{bb}")
                    nc.vector.reciprocal(rden[:], den_eps[:])
                    out_sb = small.tile([1, D], f32)
                    nc.scalar.activation(
                        out=out_sb[:], in_=pool_ps[:],
                        func=mybir.ActivationFunctionType.Copy,
                        scale=rden[0:1, 0:1])
                    nc.sync.dma_start(out=out[bb:bb + 1, :], in_=out_sb[:])

            for k in range(min(LOOKAHEAD, NQT)):
                issue_quarter(k)

            for k in range(NQT):
                b, qi = divmod(k, NQ)
                if qi == 0:
                    state[b] = {
                        "scores": small.tile([P, CHUNKS], f32, tag="scores",
                                             name=f"scores{b}"),
                        "wsum4": small.tile([P, NQ], f32, tag="wsum4",
                                            name=f"wsum4_{b}"),
                        "pool_ps": psums.tile([1, D], f32, tag="pool",
                                              name=f"pool_ps{b}"),
                        "mask_f": small.tile([P, CHUNKS], f32, tag="mask_f",
                                             name=f"mask_f{b}"),
                    }
                st = state[b]
                xq = xq_tiles.pop(k)

                # convert quarter k (if not already) and quarter k+1 ahead of
                # the STT stream so ScalarE's convert never waits behind exp
                if k == 0:
                    xh0 = xf16p.tile([P, QC, D], f16, tag="xh", name="xh0")
                    nc.scalar.copy(xh0[:], xq[:])
                    xh_tiles[0] = xh0
                xh = xh_tiles.pop(k)
                if k + 1 < NQT:
                    xh1 = xf16p.tile([P, QC, D], f16, tag="xh",
                                     name=f"xh{k + 1}")
                    nc.scalar.copy(xh1[:], xq_tiles[k + 1][:])
                    xh_tiles[k + 1] = xh1

                noff_k = 0 if k == NQT - 1 else NOFF
                for i in range(QC):
                    col = st["scores"][:, qi * QC + i:qi * QC + i + 1]
                    if i < QC - noff_k:
                        scr = scratchp.tile([P, D], f32, tag="scr")
                        nc.vector.scalar_tensor_tensor(
                            out=scr[:],
                            in0=xq[:, i, :],
                            scalar=SCALE,
                            in1=qb[:],
                            op0=mybir.AluOpType.mult,
                            op1=mybir.AluOpType.mult,
                            accum_out=col,
                        )
                    else:
                        # offload: fp16 product at DVE 2x rate, scaled
                        # reduce on ScalarE via activation accum
                        prodh = scratchp.tile([P, D], f16, tag="prodh",
                                              bufs=3)
                        nc.vector.tensor_tensor(
                            out=prodh[:], in0=xh[:, i, :], in1=qh[:],
                            op=mybir.AluOpType.mult)
                        scrh = scratchp.tile([P, D], f16, tag="scrh")
                        nc.scalar.activation(
                            out=scrh[:], in_=prodh[:],
                            func=mybir.ActivationFunctionType.Copy,
                            scale=SCALE, accum_out=col)

                if b > 0 and qi == 0:
                    flush_epilogue()
                if k + LOOKAHEAD < NQT:
                    issue_quarter(k + LOOKAHEAD)
                if qi == 0:
                    # mask cast deferred past the STT stream so the DVE's
                    # quarter start never waits on the mask DMA
                    nc.vector.tensor_copy(st["mask_f"][:],
                                          mask_tiles[b][:])

                # per-quarter softmax tail: exp, mask, fp16 weights
                sl = slice(qi * QC, (qi + 1) * QC)
                expq = small.tile([P, QC], f32, tag="expq")
                nc.scalar.activation(out=expq[:], in_=st["scores"][:, sl],
                                     func=mybir.ActivationFunctionType.Exp)
                wqh = small.tile([P, QC], f16, tag="wqh")
                nc.vector.scalar_tensor_tensor(
                    out=wqh[:], in0=expq[:], scalar=1.0,
                    in1=st["mask_f"][:, sl],
                    op0=mybir.AluOpType.mult, op1=mybir.AluOpType.mult,
                    accum_out=st["wsum4"][:, qi:qi + 1],
                )

                for i in range(QC):
                    nc.tensor.matmul(
                        st["pool_ps"][:],
                        wqh[:, i:i + 1],
                        xh[:, i, :],
                        start=(qi == 0 and i == 0),
                        stop=(qi == NQ - 1 and i == QC - 1),
                    )

                if qi == NQ - 1:
                    epilogue.append((st["pool_ps"], st["wsum4"], b))
                    del state[b]

            flush_epilogue()

    nc.compile()
    return nc


def kernel(x: np.ndarray, mask: np.ndarray, query: np.ndarray) -> np.ndarray:
    from concourse.bass_utils import run_bass_kernel_spmd

    if "nc" not in _cache:
        _cache["nc"] = _build()
    nc = _cache["nc"]

    x = np.ascontiguousarray(np.asarray(x, dtype=np.float32))
    mask = np.ascontiguousarray(np.asarray(mask, dtype=np.int32))
    query = np.ascontiguousarray(np.asarray(query, dtype=np.float32))

    in_maps = [
        {
            "x": np.ascontiguousarray(x[c * BPC:(c + 1) * BPC]),
            "mask": np.ascontiguousarray(mask[c * BPC:(c + 1) * BPC]),
            "query": query,
        }
        for c in range(N_CORES)
    ]
    res = run_bass_kernel_spmd(nc, in_maps, core_ids=list(range(N_CORES)))
    return np.concatenate([res.results[c]["out"] for c in range(N_CORES)], axis=0)
